# revision 9
# baseline (speedup 1.0000x reference)
"""VQ codebook forward (nearest-code lookup) on Trainium2 — Bass/Tile kernel.

Data-parallel over 8 NeuronCores: x [16,1024,256] is sharded along batch
(2 batches = 2048 tokens per core); the [1024,256] codebook is replicated.

Per core:
  - Convert x (pre-scaled by 2 in the fp16 rounding) and E to fp16 (hi, lo)
    pairs in native layout: hi = fp16(v), lo = fp16(v - hi).
  - Transpose all fp16 operands with the DMA crossbar (dma_start_transpose,
    128x128 fp16 blocks) so the contraction dim d sits on partitions; the
    TensorE does no transposes at all.
  - A 3-pass fp16 matmul (hi*Hi + hi*Lo + lo*Hi) reproduces the fp32 product
    to ~1e-5 — measured more accurate than a numpy fp32 matmul — at 4x the
    throughput of the PE's native fp32 LOW_HIGH mode.
  - negesq_rep[p, n] = -sum_d E[n,d]^2 for all p, via one fp32 matmul with a
    [-1]*128x128 stationary over (ETh+ETl)^2.
  - Per 128-token tile: PSUM accumulates 2*x.e via 12 fp16 matmuls; a custom
    DVE op (body Src0+Src1, max-accum) adds negesq_rep and max-reduces in one
    pass (nscore = 2*x.e - e_sq; argmin ||x-e||^2 == argmax nscore), then
    FIND_INDEX8 (max_index) returns the first index equal to the max,
    matching jnp.argmin's first-occurrence tie rule.
  - GPSIMD indirect DMA gathers E[idx] rows; DMA writes them to the output.
    Forward STE output x + q - stop_grad(x) == q up to ~1e-7 relative.
"""

from contextlib import ExitStack

import numpy as np

B, S, D = 16, 1024, 256
N_CODES = 1024
N_CORES = 8
P = 128
TOK_PER_CORE = B * S // N_CORES  # 2048
N_TILES = TOK_PER_CORE // P  # 16
HALF = 512  # codes per PSUM bank (fp32)
XG = N_TILES // 4  # x token-tile groups of 4 (512 cols per transposed group)
EG = (N_CODES // P) // 4  # codebook tile groups of 4

_CACHE = {}


def _register_addmax():
    """Custom DVE op: out = in0 + in1, accum_out = max(s0, max(out))."""
    import concourse.dve_ops as dops
    from concourse.dve_spec import C0, Spec, Src0, Src1, _has_src1, lower, maxx
    from concourse.dve_uop import DveOpSpec

    name = "VQ_TT_ADD_MAX"
    for o in dops.OPS:
        if o.name == name:
            return o

    def _ref(in0, in1, s0, s1, imm2):
        b = (in0.astype(np.float32) + in1).astype(np.float32)
        acc = np.maximum(b.reshape(b.shape[0], -1).max(-1, keepdims=True), s0)
        return b, acc

    spec = Spec(body=Src0 + Src1, accum=maxx, accum_init=C0, reference=_ref)
    opcode = dops._CUSTOM_DVE_ROW_BASE + len(dops.OPS)
    assert opcode < 0x20
    shas = {}
    for ver in ("v3", "v4"):
        uops = lower(spec, ver=ver)
        shas[ver] = DveOpSpec(
            name=name, opcode=opcode, uops=uops, rd1_en=_has_src1(spec)
        ).sha(ver)
    op = dops.DveOp(name, spec, subdim=False, uops_sha=shas)
    dops._SUB_OPCODE_FOR_NAME[name] = opcode
    dops.OPS.append(op)
    dops.CUSTOM_DVE_SPECS[name] = spec
    return op


def _build_nc():
    import concourse.bass as bass
    import concourse.mybir as mybir
    import concourse.tile as tile
    from concourse import bacc
    from concourse._compat import get_trn_type

    f32 = mybir.dt.float32
    f16 = mybir.dt.float16
    COPY = mybir.ActivationFunctionType.Copy
    addmax = _register_addmax()

    nc = bacc.Bacc(get_trn_type() or "TRN2", target_bir_lowering=False, debug=False)

    x_d = nc.dram_tensor("x", [TOK_PER_CORE, D], f32, kind="ExternalInput")
    e_d = nc.dram_tensor("embedding_weight", [N_CODES, D], f32, kind="ExternalInput")
    o_d = nc.dram_tensor("out", [TOK_PER_CORE, D], f32, kind="ExternalOutput")

    with ExitStack() as ctx:
        tc = ctx.enter_context(tile.TileContext(nc))
        singles = ctx.enter_context(tc.tile_pool(name="singles", bufs=1))

        negones = singles.tile([P, P], f32)
        nc.vector.memset(negones[:], -1.0)

        # ---- codebook: load, fp16-split, DMA-transpose ----
        # eth/etl[c][h][d, n'] = hi/lo fp16 of E[4h*128+k*128+p, c*128+d]
        e_nat = singles.tile([P, N_CODES // P, D], f32)
        eh_nat = singles.tile([P, N_CODES // P, D], f16)
        el_nat = singles.tile([P, N_CODES // P, D], f16)
        eth = [
            [singles.tile([P, HALF], f16, name=f"eth{c}_{g}") for g in range(EG)]
            for c in range(2)
        ]
        etl = [
            [singles.tile([P, HALF], f16, name=f"etl{c}_{g}") for g in range(EG)]
            for c in range(2)
        ]
        for g in range(EG):
            gs = slice(4 * g, 4 * g + 4)
            for k in range(4):
                t = 4 * g + k
                nc.sync.dma_start(
                    out=e_nat[:, t, :], in_=e_d[t * P : (t + 1) * P, :]
                )
            nc.scalar.activation(out=eh_nat[:, gs, :], in_=e_nat[:, gs, :], func=COPY)
            nc.gpsimd.tensor_tensor(
                out=el_nat[:, gs, :],
                in0=e_nat[:, gs, :],
                in1=eh_nat[:, gs, :],
                op=mybir.AluOpType.subtract,
            )
            for k in range(4):
                t = 4 * g + k
                for c in range(2):
                    cs = slice(c * P, (c + 1) * P)
                    ks = slice(k * P, (k + 1) * P)
                    nc.sync.dma_start_transpose(
                        out=eth[c][g][:, ks], in_=eh_nat[:, t, cs]
                    )
                    nc.sync.dma_start_transpose(
                        out=etl[c][g][:, ks], in_=el_nat[:, t, cs]
                    )

        # ---- x shard: load, fp16-split of 2*x, DMA-transpose ----
        x_nat = singles.tile([P, N_TILES, D], f32)
        xh_nat = singles.tile([P, N_TILES, D], f16)
        xl_nat = singles.tile([P, N_TILES, D], f16)
        xth = [
            [singles.tile([P, HALF], f16, name=f"xth{c}_{g}") for g in range(XG)]
            for c in range(2)
        ]
        xtl = [
            [singles.tile([P, HALF], f16, name=f"xtl{c}_{g}") for g in range(XG)]
            for c in range(2)
        ]
        for g in range(XG):
            gs = slice(4 * g, 4 * g + 4)
            for k in range(4):
                i = 4 * g + k
                nc.sync.dma_start(
                    out=x_nat[:, i, :], in_=x_d[i * P : (i + 1) * P, :]
                )
            # hi = fp16(2*x); lo = fp16(2*x - hi)
            nc.scalar.activation(
                out=xh_nat[:, gs, :], in_=x_nat[:, gs, :], func=COPY, scale=2.0
            )
            nc.vector.scalar_tensor_tensor(
                out=xl_nat[:, gs, :],
                in0=x_nat[:, gs, :],
                scalar=2.0,
                in1=xh_nat[:, gs, :],
                op0=mybir.AluOpType.mult,
                op1=mybir.AluOpType.subtract,
            )
            for k in range(4):
                i = 4 * g + k
                for c in range(2):
                    cs = slice(c * P, (c + 1) * P)
                    ks = slice(k * P, (k + 1) * P)
                    nc.sync.dma_start_transpose(
                        out=xth[c][g][:, ks], in_=xh_nat[:, i, cs]
                    )
                    nc.sync.dma_start_transpose(
                        out=xtl[c][g][:, ks], in_=xl_nat[:, i, cs]
                    )

        # ---- negesq_rep[p, n] = -sum_d E[n,d]^2 for every partition p ----
        # ET^2 rebuilt from the fp16 pair: (eth+etl)^2 == ET^2 to ~2^-22.
        et2 = [
            [singles.tile([P, HALF], f32, name=f"et2_{c}_{g}") for g in range(EG)]
            for c in range(2)
        ]
        for c in range(2):
            for g in range(EG):
                nc.gpsimd.tensor_tensor(
                    out=et2[c][g][:],
                    in0=eth[c][g][:],
                    in1=etl[c][g][:],
                    op=mybir.AluOpType.add,
                )
                nc.scalar.square(out=et2[c][g][:], in_=et2[c][g][:])
        negesq = singles.tile([P, N_CODES], f32)
        with tc.tile_pool(name="psum_esq", bufs=1, space="PSUM") as psum_esq:
            esq_ps = psum_esq.tile([P, N_CODES], f32)
            for h in range(2):
                cols = slice(h * HALF, (h + 1) * HALF)
                for c in range(2):
                    nc.tensor.matmul(
                        out=esq_ps[:, cols],
                        lhsT=negones[:],
                        rhs=et2[c][h][:],
                        start=(c == 0),
                        stop=(c == 1),
                    )
            nc.scalar.activation(out=negesq[:], in_=esq_ps[:], func=COPY)

        # ---- main loop over 16 token tiles ----
        work = ctx.enter_context(tc.tile_pool(name="work", bufs=3))
        outp = ctx.enter_context(tc.tile_pool(name="outp", bufs=3))
        psum_s = ctx.enter_context(tc.tile_pool(name="psum_s", bufs=3, space="PSUM"))
        h_slices = [slice(0, HALF), slice(HALF, N_CODES)]
        for i in range(N_TILES):
            g, k = i // 4, i % 4
            ks = slice(k * P, (k + 1) * P)
            ps = psum_s.tile([P, N_CODES], f32)
            # stationary-major order: 4 weight loads, 12 matmuls
            plan = [
                (xth[0][g], [eth[0], etl[0]]),
                (xth[1][g], [eth[1], etl[1]]),
                (xtl[0][g], [eth[0]]),
                (xtl[1][g], [eth[1]]),
            ]
            n_done = [0, 0]
            for stat, rhs_list in plan:
                for rhs in rhs_list:
                    for h in range(2):
                        nc.tensor.matmul(
                            out=ps[:, h_slices[h]],
                            lhsT=stat[:, ks],
                            rhs=rhs[h][:],
                            start=(n_done[h] == 0),
                            stop=(n_done[h] == 5),
                        )
                        n_done[h] += 1

            # nscore = psum + negesq_rep -> SBUF; fused max-accum -> needle
            score = work.tile([P, N_CODES], f32)
            m8 = work.tile([P, 8], f32)
            nc.vector._custom_dve(
                addmax,
                out=score[:],
                in0=ps[:],
                in1=negesq[:],
                s0=-3.0e38,
                accum_out=m8[:, 0:1],
            )
            idx8 = work.tile([P, 8], mybir.dt.uint32)
            nc.vector.max_index(out=idx8[:], in_max=m8[:], in_values=score[:])

            q = outp.tile([P, D], f32)
            nc.gpsimd.indirect_dma_start(
                out=q[:],
                out_offset=None,
                in_=e_d[:],
                in_offset=bass.IndirectOffsetOnAxis(ap=idx8[:, 0:1], axis=0),
            )
            nc.sync.dma_start(out=o_d[i * P : (i + 1) * P, :], in_=q[:])

    nc.finalize()
    return nc


def _get_nc():
    if "nc" not in _CACHE:
        _CACHE["nc"] = _build_nc()
    return _CACHE["nc"]


def run(inputs, trace=False):
    """Run on all 8 cores. Returns (full_output [16,1024,256] f32, exec_time_ns)."""
    from concourse.bass_utils import run_bass_kernel_spmd

    nc = _get_nc()
    x = np.ascontiguousarray(np.asarray(inputs["x"], dtype=np.float32)).reshape(
        B * S, D
    )
    e = np.ascontiguousarray(np.asarray(inputs["embedding_weight"], dtype=np.float32))
    shards = x.reshape(N_CORES, TOK_PER_CORE, D)
    in_maps = [
        {"x": np.ascontiguousarray(shards[c]), "embedding_weight": e}
        for c in range(N_CORES)
    ]
    res = run_bass_kernel_spmd(
        nc, in_maps, core_ids=list(range(N_CORES)), trace=trace
    )
    out = np.concatenate([r["out"] for r in res.results], axis=0).reshape(B, S, D)
    return out, res.exec_time_ns


def kernel(x, embedding_weight):
    out, _ = run({"x": x, "embedding_weight": embedding_weight})
    return out


# revision 12
# speedup vs baseline: 2.4651x; 2.4651x over previous
"""VQ codebook forward (nearest-code lookup) on Trainium2 — Bass/Tile kernel.

Data-parallel over 8 NeuronCores: x [16,1024,256] is sharded along batch
(2 batches = 2048 tokens per core); the [1024,256] codebook is replicated.

Per core:
  - TensorE identity-matmul transposes put the contraction dim d on
    partitions (PSUM); the PSUM->SBUF copies split straight into fp16
    (hi, lo) pairs: ScalarE writes hi = fp16(v) (x side scaled by 2),
    VectorE scalar_tensor_tensor writes lo = fp16(v - hi). No fp32
    transposed intermediates exist.
  - A 3-pass fp16 matmul (hi*Hi + hi*Lo + lo*Hi) reproduces the fp32 product
    to ~1e-5 — measured more accurate than a numpy fp32 matmul — at 4x the
    throughput of the PE's native fp32 LOW_HIGH mode.
  - negesq_rep[p, n] = -sum_d E[n,d]^2 for all p, via one fp32 matmul with a
    [-1]*128x128 stationary over (ETh+ETl)^2.
  - Per 128-token tile: PSUM accumulates 2*x.e via 12 fp16 matmuls; a custom
    DVE op (body Src0+Src1, max-accum) adds negesq_rep and max-reduces in one
    pass (nscore = 2*x.e - e_sq; argmin ||x-e||^2 == argmax nscore), then
    FIND_INDEX8 (max_index) returns the first index equal to the max,
    matching jnp.argmin's first-occurrence tie rule.
  - GPSIMD indirect DMA gathers E[idx] rows; DMA writes them to the output.
    Forward STE output x + q - stop_grad(x) == q up to ~1e-7 relative.
"""

from contextlib import ExitStack

import numpy as np

B, S, D = 16, 1024, 256
N_CODES = 1024
N_CORES = 8
P = 128
TOK_PER_CORE = B * S // N_CORES  # 2048
N_TILES = TOK_PER_CORE // P  # 16
HALF = 512  # codes per PSUM bank (fp32)
XG = N_TILES // 4  # x transpose groups (4 token tiles -> 512 cols each)
EG = (N_CODES // P) // 4  # codebook transpose groups

_CACHE = {}


def _register_addmax():
    """Custom DVE op: out = in0 + in1, accum_out = max(s0, max(out))."""
    import concourse.dve_ops as dops
    from concourse.dve_spec import C0, Spec, Src0, Src1, _has_src1, lower, maxx
    from concourse.dve_uop import DveOpSpec

    name = "VQ_TT_ADD_MAX"
    for o in dops.OPS:
        if o.name == name:
            return o

    def _ref(in0, in1, s0, s1, imm2):
        b = (in0.astype(np.float32) + in1).astype(np.float32)
        acc = np.maximum(b.reshape(b.shape[0], -1).max(-1, keepdims=True), s0)
        return b, acc

    spec = Spec(body=Src0 + Src1, accum=maxx, accum_init=C0, reference=_ref)
    opcode = dops._CUSTOM_DVE_ROW_BASE + len(dops.OPS)
    assert opcode < 0x20
    shas = {}
    for ver in ("v3", "v4"):
        uops = lower(spec, ver=ver)
        shas[ver] = DveOpSpec(
            name=name, opcode=opcode, uops=uops, rd1_en=_has_src1(spec)
        ).sha(ver)
    op = dops.DveOp(name, spec, subdim=False, uops_sha=shas)
    dops._SUB_OPCODE_FOR_NAME[name] = opcode
    dops.OPS.append(op)
    dops.CUSTOM_DVE_SPECS[name] = spec
    return op


def _make_identity(nc, mybir, ident):
    nc.gpsimd.memset(ident[:], 0.0)
    nc.gpsimd.affine_select(
        out=ident[:],
        in_=ident[:],
        compare_op=mybir.AluOpType.not_equal,
        fill=1.0,
        base=0,
        pattern=[[-1, P]],
        channel_multiplier=1,
    )


def _build_nc():
    import concourse.bass as bass
    import concourse.mybir as mybir
    import concourse.tile as tile
    from concourse import bacc
    from concourse._compat import get_trn_type

    f32 = mybir.dt.float32
    f16 = mybir.dt.float16
    COPY = mybir.ActivationFunctionType.Copy
    addmax = _register_addmax()

    nc = bacc.Bacc(get_trn_type() or "TRN2", target_bir_lowering=False, debug=False)

    x_d = nc.dram_tensor("x", [TOK_PER_CORE, D], f32, kind="ExternalInput")
    e_d = nc.dram_tensor("embedding_weight", [N_CODES, D], f32, kind="ExternalInput")
    o_d = nc.dram_tensor("out", [TOK_PER_CORE, D], f32, kind="ExternalOutput")

    with ExitStack() as ctx:
        tc = ctx.enter_context(tile.TileContext(nc))
        singles = ctx.enter_context(tc.tile_pool(name="singles", bufs=1))

        ident = singles.tile([P, P], f32)
        _make_identity(nc, mybir, ident)
        negones = singles.tile([P, P], f32)
        nc.vector.memset(negones[:], -1.0)

        e_nat = singles.tile([P, N_CODES // P, D], f32)
        x_nat = singles.tile([P, N_TILES, D], f32)
        eth = [
            [singles.tile([P, HALF], f16, name=f"eth{c}_{g}") for g in range(EG)]
            for c in range(2)
        ]
        etl = [
            [singles.tile([P, HALF], f16, name=f"etl{c}_{g}") for g in range(EG)]
            for c in range(2)
        ]
        xth = [
            [singles.tile([P, HALF], f16, name=f"xth{c}_{g}") for g in range(XG)]
            for c in range(2)
        ]
        xtl = [
            [singles.tile([P, HALF], f16, name=f"xtl{c}_{g}") for g in range(XG)]
            for c in range(2)
        ]

        setup_ctx = ExitStack()
        psum_tr = setup_ctx.enter_context(
            tc.tile_pool(name="psum_tr", bufs=2, space="PSUM")
        )

        # ---- codebook: load, transpose, split to fp16 hi/lo during copy-out ----
        for g in range(EG):
            for k in range(4):
                t = 4 * g + k
                nc.sync.dma_start(out=e_nat[:, t, :], in_=e_d[t * P : (t + 1) * P, :])
            for c in range(2):
                pt = psum_tr.tile([P, HALF], f32, name=f"pt_e{c}_{g}", tag="pt")
                for k in range(4):
                    t = 4 * g + k
                    nc.tensor.transpose(
                        out=pt[:, k * P : (k + 1) * P],
                        in_=e_nat[:, t, c * P : (c + 1) * P],
                        identity=ident[:],
                    )
                nc.scalar.activation(out=eth[c][g][:], in_=pt[:], func=COPY)
                nc.vector.scalar_tensor_tensor(
                    out=etl[c][g][:],
                    in0=pt[:],
                    scalar=1.0,
                    in1=eth[c][g][:],
                    op0=mybir.AluOpType.mult,
                    op1=mybir.AluOpType.subtract,
                )

        # ---- x shard: load, transpose, split fp16 hi/lo of 2*x during copy ----
        for g in range(XG):
            for k in range(4):
                i = 4 * g + k
                nc.sync.dma_start(out=x_nat[:, i, :], in_=x_d[i * P : (i + 1) * P, :])
            for c in range(2):
                pt = psum_tr.tile([P, HALF], f32, name=f"pt_x{c}_{g}", tag="pt")
                for k in range(4):
                    i = 4 * g + k
                    nc.tensor.transpose(
                        out=pt[:, k * P : (k + 1) * P],
                        in_=x_nat[:, i, c * P : (c + 1) * P],
                        identity=ident[:],
                    )
                nc.scalar.activation(
                    out=xth[c][g][:], in_=pt[:], func=COPY, scale=2.0
                )
                nc.vector.scalar_tensor_tensor(
                    out=xtl[c][g][:],
                    in0=pt[:],
                    scalar=2.0,
                    in1=xth[c][g][:],
                    op0=mybir.AluOpType.mult,
                    op1=mybir.AluOpType.subtract,
                )

        # ---- negesq_rep[p, n] = -sum_d E[n,d]^2 for every partition p ----
        et2 = [
            [singles.tile([P, HALF], f32, name=f"et2_{c}_{g}") for g in range(EG)]
            for c in range(2)
        ]
        for c in range(2):
            for g in range(EG):
                nc.gpsimd.tensor_tensor(
                    out=et2[c][g][:],
                    in0=eth[c][g][:],
                    in1=etl[c][g][:],
                    op=mybir.AluOpType.add,
                )
                nc.scalar.square(out=et2[c][g][:], in_=et2[c][g][:])
        negesq = singles.tile([P, N_CODES], f32)
        with tc.tile_pool(name="psum_esq", bufs=1, space="PSUM") as psum_esq:
            esq_ps = psum_esq.tile([P, N_CODES], f32)
            for h in range(2):
                cols = slice(h * HALF, (h + 1) * HALF)
                for c in range(2):
                    nc.tensor.matmul(
                        out=esq_ps[:, cols],
                        lhsT=negones[:],
                        rhs=et2[c][h][:],
                        start=(c == 0),
                        stop=(c == 1),
                    )
            nc.scalar.activation(out=negesq[:], in_=esq_ps[:], func=COPY)
        setup_ctx.close()

        # ---- main loop over 16 token tiles ----
        work = ctx.enter_context(tc.tile_pool(name="work", bufs=3))
        outp = ctx.enter_context(tc.tile_pool(name="outp", bufs=3))
        psum_s = ctx.enter_context(tc.tile_pool(name="psum_s", bufs=3, space="PSUM"))
        h_slices = [slice(0, HALF), slice(HALF, N_CODES)]
        for i in range(N_TILES):
            g, k = i // 4, i % 4
            ks = slice(k * P, (k + 1) * P)
            ps = psum_s.tile([P, N_CODES], f32)
            # stationary-major order: 4 weight loads, 12 matmuls
            plan = [
                (xth[0][g], [eth[0], etl[0]]),
                (xth[1][g], [eth[1], etl[1]]),
                (xtl[0][g], [eth[0]]),
                (xtl[1][g], [eth[1]]),
            ]
            n_done = [0, 0]
            for stat, rhs_list in plan:
                for rhs in rhs_list:
                    for h in range(2):
                        nc.tensor.matmul(
                            out=ps[:, h_slices[h]],
                            lhsT=stat[:, ks],
                            rhs=rhs[h][:],
                            start=(n_done[h] == 0),
                            stop=(n_done[h] == 5),
                        )
                        n_done[h] += 1

            # nscore = psum + negesq_rep -> SBUF; fused max-accum -> needle
            score = work.tile([P, N_CODES], f32)
            m8 = work.tile([P, 8], f32)
            nc.vector._custom_dve(
                addmax,
                out=score[:],
                in0=ps[:],
                in1=negesq[:],
                s0=-3.0e38,
                accum_out=m8[:, 0:1],
            )
            idx8 = work.tile([P, 8], mybir.dt.uint32)
            nc.vector.max_index(out=idx8[:], in_max=m8[:], in_values=score[:])

            q = outp.tile([P, D], f32)
            nc.gpsimd.indirect_dma_start(
                out=q[:],
                out_offset=None,
                in_=e_d[:],
                in_offset=bass.IndirectOffsetOnAxis(ap=idx8[:, 0:1], axis=0),
            )
            nc.sync.dma_start(out=o_d[i * P : (i + 1) * P, :], in_=q[:])

    nc.finalize()
    return nc


def _get_nc():
    if "nc" not in _CACHE:
        _CACHE["nc"] = _build_nc()
    return _CACHE["nc"]


def run(inputs, trace=False):
    """Run on all 8 cores. Returns (full_output [16,1024,256] f32, exec_time_ns)."""
    from concourse.bass_utils import run_bass_kernel_spmd

    nc = _get_nc()
    x = np.ascontiguousarray(np.asarray(inputs["x"], dtype=np.float32)).reshape(
        B * S, D
    )
    e = np.ascontiguousarray(np.asarray(inputs["embedding_weight"], dtype=np.float32))
    shards = x.reshape(N_CORES, TOK_PER_CORE, D)
    in_maps = [
        {"x": np.ascontiguousarray(shards[c]), "embedding_weight": e}
        for c in range(N_CORES)
    ]
    res = run_bass_kernel_spmd(
        nc, in_maps, core_ids=list(range(N_CORES)), trace=trace
    )
    out = np.concatenate([r["out"] for r in res.results], axis=0).reshape(B, S, D)
    return out, res.exec_time_ns


def kernel(x, embedding_weight):
    out, _ = run({"x": x, "embedding_weight": embedding_weight})
    return out
